# revision 17
# baseline (speedup 1.0000x reference)
"""Trainium2 Bass kernel for nn_BackProject: batched bilinear sampling.

reference: out[b, d, h, w, c] = bilinear_sample(inputs[b], coords[b, d, h, w])
  inputs [2, 120, 160, 32] f32, coords [2, 32, 120, 160, 2] f32 (x, y),
  out [2, 32, 120, 160, 32] f32.

Sharding: 64 (b, d) planes / 8 cores = 8 planes per core; cores 0-3 take
b=0, cores 4-7 take b=1. Each core holds the full [H, W, C] feature map.

Device algorithm (per core):
  1. Build a "quad table" qt[p] = pixels [p, p+1, p+W, p+W+1] (512 B rows)
     in DRAM: 4 shifted contiguous loads of the (host-padded) feature map
     into SBUF, DVE-interleave, one fat contiguous store.  Since
     x in [0, W-1) and y in [0, H-1), the 4 bilinear taps of a sample at
     (x, y) are exactly row y0*W+x0 of the quad table (no clipping).
  2. Compute int16 gather indices y0*W+x0 on DVE directly in the wrapped
     [16, n/16] layout dma_gather wants (8 planes batched across the 128
     partitions), then replicate 8x across partition groups (HW Q7 cores
     each read their own 16-partition window).
  3. One dma_gather per (plane, half-plane): 9600 indices x 512 B quads
     into SBUF tiles [128, 75, 128], alternating between 2 SWDGE queues
     (descriptor processing, not HBM bandwidth, is the gather bottleneck).
  4. Bilinear weights from a magic-number floor on DVE; 7 tensor_tensor
     passes (4 mul + 3 add) with free-dim-broadcast weights.
  5. Store via the scalar-engine HWDGE ring (loads use the SP ring).
"""

import sys

for _p in ("/opt/trn_rl_repo", "/opt/pypackages"):
    if _p not in sys.path:
        sys.path.append(_p)

import numpy as np

B, H, W, C = 2, 120, 160, 32
D = 32
P = H * W            # 19200 positions per plane
PLANES = 8           # planes per core
HALF = P // 2        # 9600 positions per gather
S = 75               # gather-tile columns (positions per partition per half)
QROWS = P - W - 1    # 19039 valid quad rows (max gathered idx is 19038)
FPAD = 19440         # host-padded feature-map rows (shifted-load AP windows)
MAGIC = 12582912.0   # 1.5 * 2**23: x + MAGIC - MAGIC == rne(x) for 0<=x<2**22

_cache = {}


def _split_multi_waits(nc):
    """The pinned walrus build accepts only one sync-wait per instruction;
    Tile aggregates several.  Hoist all but the last wait of every
    instruction onto same-engine NOPs inserted right before it."""
    import concourse.mybir as mybir

    for bb in nc.main_func.blocks:
        lst = bb.instructions
        snapshot = list(lst)
        if not any(
            i.sync_info is not None and i.sync_info.on_wait and len(i.sync_info.on_wait) > 1
            for i in snapshot
        ):
            continue
        rebuilt = []
        for inst in snapshot:
            si = inst.sync_info
            if si is not None and si.on_wait and len(si.on_wait) > 1:
                waits = list(si.on_wait)
                eng = nc.engines[inst.engine]
                for w in waits[:-1]:
                    nop = eng.nop().ins
                    # nop() appended itself somewhere; pull it out
                    for bb2 in nc.main_func.blocks:
                        l2 = bb2.instructions
                        if l2 and l2[-1] is nop:
                            l2.remove(nop)
                            break
                    nop.sync_info = mybir.SyncInfo(on_wait=[w], on_update=[])
                    rebuilt.append(nop)
                si.on_wait = waits[-1:]
            rebuilt.append(inst)
        lst.clear()
        lst.extend(rebuilt)


def _build():
    import concourse.bass as bass
    import concourse.mybir as mybir
    import concourse.tile as tile
    from concourse import library_config
    from concourse.library_overlay import lower_extended_insts
    from bass_rust import add_dep_helper

    f32 = mybir.dt.float32
    i16 = mybir.dt.int16
    Alu = mybir.AluOpType

    nc = bass.Bass(num_swdge_queues=2)
    fmap = nc.dram_tensor("fmap", [FPAD, C], f32, kind="ExternalInput")
    coords = nc.dram_tensor("coords", [PLANES, P, 2], f32, kind="ExternalInput")
    out = nc.dram_tensor("out", [PLANES, P, C], f32, kind="ExternalOutput")

    with tile.TileContext(nc) as tc:
        with (
            tc.tile_pool(name="dram", bufs=1, space="DRAM") as dpool,
            tc.tile_pool(name="persist", bufs=1) as pers,
        ):
            ll = nc.gpsimd.load_library(library_config.mlp)
            gathers = []
            v = nc.vector

            # --- Phase A: quad table via SBUF interleave ---------------------
            # qt row p = (y0, x0): pixels [p, p+1, p+W, p+W+1].  Partition Pn
            # owns quads [150*Pn, 150*Pn+150), split into 2 chunks of 75.
            qt = dpool.tile([128 * 150, 4 * C], f32)
            with tc.tile_pool(name="qbuild", bufs=2) as qb:
                # DVE warmup: first DVE op pays a ~50us microcode table load;
                # do it here so it overlaps the quad build.
                warm = qb.tile([128, 8], f32, tag="warm")
                v.memset(warm[:], 0.0)
                v.tensor_scalar_add(warm[:], warm[:], 1.0)
                for c in range(2):
                    qtile = qb.tile([128, S, 4 * C], f32, tag="qtile")
                    for k, off in enumerate((0, 1, W, W + 1)):
                        ft = qb.tile([128, S, C], f32, tag=f"ft{k}")
                        # pixels [150*Pn + 75c + off + t] for t in [0, 75)
                        src = fmap.rearrange("q c -> (q c)")[
                            bass.ds(C * (S * c + off), 128 * 150 * C)
                        ].rearrange("(p t c) -> p t c", p=128, t=150, c=C)
                        nc.sync.dma_start(ft[:], src[:, 0:S, :])
                        v.tensor_copy(qtile[:, :, k * C:(k + 1) * C], ft[:])
                    nc.scalar.dma_start(
                        qt.rearrange("(p x t) c -> p x (t c)", p=128, x=2, t=S)[:, c],
                        qtile[:].rearrange("p t c -> p (t c)"),
                    )

            # --- Phase B: gather indices in wrapped layout -------------------
            # Gather (d, h) list-pos j = tt*128 + p maps to plane position
            # 9600h + j, so gt[p, tt] = quad(pos 9600h + tt*128 + p).  The idx
            # for list-pos j sits at wrapped [j%16, j//16]; batched over
            # planes: cw[16d+r, u, e] = coords[d, 16u+r, e].
            pidx = pers.tile([128, PLANES * 1200], i16)
            with tc.tile_pool(name="idxb", bufs=1) as ib:
                cw = ib.tile([128, 1200, 2], f32)
                for d in range(PLANES):
                    nc.sync.dma_start(
                        cw[16 * d:16 * (d + 1)],
                        coords[d].rearrange("(u r) e -> r u e", u=1200, r=16),
                    )

                xw = cw[:, :, 0]
                yw = cw[:, :, 1]
                rx = ib.tile([128, 1200], f32)
                ry = ib.tile([128, 1200], f32)
                gtw = ib.tile([128, 1200], f32)
                pixf = ib.tile([128, 1200], f32)
                idx16 = ib.tile([128, 1200], i16)
                # floor(x) via round-to-nearest, correcting the x < rne(x) case
                v.tensor_scalar_add(rx[:], xw, MAGIC)
                v.tensor_scalar_add(rx[:], rx[:], -MAGIC)
                v.tensor_tensor(gtw[:], rx[:], xw, Alu.is_gt)
                v.tensor_tensor(rx[:], rx[:], gtw[:], Alu.subtract)  # x0f
                v.tensor_scalar_add(ry[:], yw, MAGIC)
                v.tensor_scalar_add(ry[:], ry[:], -MAGIC)
                v.tensor_tensor(gtw[:], ry[:], yw, Alu.is_gt)
                v.tensor_tensor(ry[:], ry[:], gtw[:], Alu.subtract)  # y0f
                v.tensor_scalar_mul(pixf[:], ry[:], float(W))
                v.tensor_tensor(pixf[:], pixf[:], rx[:], Alu.add)
                v.tensor_copy(idx16[:], pixf[:])  # f32 -> int16 (exact ints)

                # replicate across the 8 gpsimd core windows
                for d in range(PLANES):
                    for g in range(8):
                        nc.sync.dma_start(
                            pidx[16 * g:16 * (g + 1), 1200 * d:1200 * (d + 1)],
                            idx16[16 * d:16 * (d + 1), :],
                        )

            # --- Phase C: per-plane weights, gather, lerp, store -------------
            with (
                tc.tile_pool(name="wts", bufs=2) as wts,
                tc.tile_pool(name="g", bufs=2) as gp,
                tc.tile_pool(name="o", bufs=2) as op_,
                tc.tile_pool(name="tmp", bufs=2) as tp,
            ):
                for d in range(PLANES):
                    # cn[p, t, e] = coords[d, t*128 + p, e] (gather order)
                    cn = wts.tile([128, 2 * S, 2], f32, tag="cn")
                    nc.sync.dma_start(
                        cn[:],
                        coords[d].rearrange("(t p) e -> p t e", t=2 * S, p=128),
                    )
                    wx = wts.tile([128, 2 * S], f32, tag="wx")
                    wy = wts.tile([128, 2 * S], f32, tag="wy")
                    omwx = wts.tile([128, 2 * S], f32, tag="omwx")
                    omwy = wts.tile([128, 2 * S], f32, tag="omwy")
                    w00 = wts.tile([128, 2 * S], f32, tag="w00")
                    w01 = wts.tile([128, 2 * S], f32, tag="w01")
                    w10 = wts.tile([128, 2 * S], f32, tag="w10")
                    w11 = wts.tile([128, 2 * S], f32, tag="w11")
                    xn = cn[:, :, 0]
                    yn = cn[:, :, 1]
                    # wx = x - floor(x)
                    v.tensor_scalar_add(omwx[:], xn, MAGIC)
                    v.tensor_scalar_add(omwx[:], omwx[:], -MAGIC)
                    v.tensor_tensor(w00[:], omwx[:], xn, Alu.is_gt)
                    v.tensor_tensor(omwx[:], omwx[:], w00[:], Alu.subtract)
                    v.tensor_tensor(wx[:], xn, omwx[:], Alu.subtract)
                    v.tensor_scalar_add(omwy[:], yn, MAGIC)
                    v.tensor_scalar_add(omwy[:], omwy[:], -MAGIC)
                    v.tensor_tensor(w00[:], omwy[:], yn, Alu.is_gt)
                    v.tensor_tensor(omwy[:], omwy[:], w00[:], Alu.subtract)
                    v.tensor_tensor(wy[:], yn, omwy[:], Alu.subtract)
                    v.tensor_scalar(omwx[:], wx[:], -1.0, 1.0, Alu.mult, Alu.add)
                    v.tensor_scalar(omwy[:], wy[:], -1.0, 1.0, Alu.mult, Alu.add)
                    v.tensor_tensor(w00[:], omwx[:], omwy[:], Alu.mult)
                    v.tensor_tensor(w01[:], wx[:], omwy[:], Alu.mult)
                    v.tensor_tensor(w10[:], omwx[:], wy[:], Alu.mult)
                    v.tensor_tensor(w11[:], wx[:], wy[:], Alu.mult)

                    for h in range(2):
                        gt = gp.tile([128, S, 4 * C], f32, tag="gt")
                        gi = nc.gpsimd.dma_gather(
                            gt[:],
                            qt[0:QROWS],
                            pidx[:, 1200 * d + 600 * h:1200 * d + 600 * (h + 1)],
                            HALF,
                            HALF,
                            4 * C,
                            single_packet=False,
                            queue_num=(2 * d + h) % 2,
                        )
                        add_dep_helper(gi.ins, ll.ins, False, "lib before gather")
                        gathers.append(gi)

                        ot = op_.tile([128, S, C], f32, tag="ot")
                        tmp = tp.tile([128, S, C], f32, tag="tmp")

                        def wb(wt):
                            return (
                                wt[:, S * h:S * (h + 1)]
                                .unsqueeze(2)
                                .broadcast_to([128, S, C])
                            )

                        v.tensor_tensor(ot[:], gt[:, :, 0:C], wb(w00), Alu.mult)
                        v.tensor_tensor(tmp[:], gt[:, :, C:2 * C], wb(w01), Alu.mult)
                        v.tensor_tensor(ot[:], ot[:], tmp[:], Alu.add)
                        v.tensor_tensor(tmp[:], gt[:, :, 2 * C:3 * C], wb(w10), Alu.mult)
                        v.tensor_tensor(ot[:], ot[:], tmp[:], Alu.add)
                        v.tensor_tensor(tmp[:], gt[:, :, 3 * C:4 * C], wb(w11), Alu.mult)
                        v.tensor_tensor(ot[:], ot[:], tmp[:], Alu.add)

                        dst = out[d].rearrange(
                            "(h t p) c -> h p t c", h=2, t=S, p=128
                        )
                        nc.scalar.dma_start(dst[h], ot[:])

    _split_multi_waits(nc)
    lower_extended_insts(nc)
    return nc


def _make_in_maps(inputs, coords):
    inputs = np.ascontiguousarray(np.asarray(inputs, dtype=np.float32))
    coords = np.ascontiguousarray(np.asarray(coords, dtype=np.float32))
    in_maps = []
    for k in range(8):
        b = k // 4
        d0 = 8 * (k % 4)
        fpad = np.zeros((FPAD, C), dtype=np.float32)
        fpad[:P] = inputs[b].reshape(P, C)
        in_maps.append(
            {
                "fmap": fpad,
                "coords": np.ascontiguousarray(
                    coords[b, d0:d0 + 8].reshape(PLANES, P, 2)
                ),
            }
        )
    return in_maps


def kernel(inputs, coords):
    if "nc" not in _cache:
        _cache["nc"] = _build()
    nc = _cache["nc"]

    from concourse.bass_utils import run_bass_kernel_spmd

    in_maps = _make_in_maps(inputs, coords)
    res = run_bass_kernel_spmd(nc, in_maps, core_ids=list(range(8)))

    out = np.empty((B, D, H, W, C), dtype=np.float32)
    for k in range(8):
        b = k // 4
        d0 = 8 * (k % 4)
        out[b, d0:d0 + 8] = res.results[k]["out"].reshape(PLANES, H, W, C)
    return out
